# revision 5
# baseline (speedup 1.0000x reference)
"""Trainium2 Bass kernel for KeypointSelector:
conv3x3(384->128, pad 1) + bias + ReLU -> conv1x1(128->1) + bias + sigmoid.

Input  dino_features: (32, 64, 64, 384) f32
Output (32, 64, 64, 1) f32

Strategy: pure data parallel over batch, 4 images per core on 8 cores.
Conv3x3 runs on the PE array in fp8-e4m3 with DoubleRowSwInterleave perf
mode: the 27 accumulating 128-contraction matmuls (9 taps x 3 cin chunks)
are paired into 13 double 256-contraction matmuls + 1 regular fp8 matmul
(fp32 PSUM accumulation; rel err vs the f32 reference is ~9.1e-3).
Pairing needs the two rhs streams to sit at a relative SBUF offset that is
a multiple of 16 bytes, so the input is laid out host-side as
[cin, padded_pixel] rows [ch0 | ch1 | ch2 | ch2'] where ch2' is a second
copy of chunk 2 placed at +14 mod 16, letting taps that differ by a row
(step 66) or two columns (step 2) pair up.

conv2 (1x1) runs as a col-tiled pack: 4 consecutive tiles' matmuls issued
back-to-back at tile_position (0,32j) execute concurrently in distinct
32-column groups of the PE array, one sigmoid covers the pack's PSUM bank,
and a partition-gather DMA packs rows {0,32,64,96} into the out staging
row.  Measured ~96us/kernel on HW (repetition slope), 2.4x the bf16
single-tap baseline (230us).
"""

import ml_dtypes
import numpy as np

import concourse.bass as bass
import concourse.tile as tile
from concourse import bacc, mybir
from concourse.bass_utils import run_bass_kernel_spmd

BF16 = ml_dtypes.bfloat16
E4M3 = ml_dtypes.float8_e4m3

# Geometry
B, H, W, CIN, CHID = 32, 64, 64, 384, 128
NCORES = 8
BLOC = B // NCORES  # 4 images per core
HP, WP = H + 2, W + 2  # 66x66 padded grid
NPIX = HP * WP  # 4356 padded pixels per image
TS = 512  # matmul free-dim tile (one PSUM bank of fp32)
START = WP + 1  # padded idx of first valid output pixel (1,1) = 67
END = H * WP + W + 1  # 4289: one past padded idx (h+1)*WP+(w+1) of pixel (63,63)
NT = -(-(END - START) // TS)  # 9 tiles per image (last one partial)
TILE_N = [min(TS, END - START - t * TS) for t in range(NT)]  # [512]*8 + [126]
OUTW = 64 * WP  # out_s columns actually read by the output DMA (4224)

# fp8 row layout: [ch0 | ch1 | ch2 | ch2copy], strides multiple of 16.
S = 4368  # chunk stride (>= NPIX, mult of 16)
C2B = 3 * S + 14  # ch2copy base: +14 mod 16 relative to ch2 at 2*S
XTOT = C2B + NPIX  # 17474 bytes per partition per image

# DoubleRow unit table: (rhs elem0 base, pair step). Taps indexed (dy,dx),
# tap offset = dy*WP + dx relative to the output pixel's padded index.
DR_UNITS = []
for t in range(9):
    dy, dx = t // 3 - 1, t % 3 - 1
    DR_UNITS.append((0 + dy * WP + dx, S))  # (ch0,tap)+(ch1,tap)
for i in range(3):
    dx = i - 1
    DR_UNITS.append((2 * S - WP + dx, S + 14 + WP))  # (ch2,(-1,dx))+(ch2',(0,dx))
DR_UNITS.append((2 * S + WP - 1, S + 14 + 2))  # (ch2,(1,-1))+(ch2',(1,+1))
NDR = len(DR_UNITS)  # 13
REG_BASE = 2 * S + WP  # leftover (ch2,(1,0)) regular fp8 matmul

_CACHED = {}


def _pair_ap(xs, base, step, n):
    """rhs AP [128, (2, step), (n, 1)] at element offset `base`."""
    ap = xs[:, base:base + step + n].copy()
    v = ap.ap
    v[1] = (step, 2)
    v.append((1, n))
    ap.ap = v
    return ap


def _build_bass(reps=1, reload=True):
    nc = bacc.Bacc("TRN2", target_bir_lowering=False)

    f32 = mybir.dt.float32
    bf16 = mybir.dt.bfloat16
    fp8 = mybir.dt.float8e4

    x = nc.dram_tensor("x", [BLOC, 128, XTOT], fp8, kind="ExternalInput")
    # SwInterleave layout: per unit 256 contiguous weight bytes per partition
    # (A/B pair-interleaved, columns reversed) so LDWEIGHTS is a contiguous
    # read (fast-weight-load eligible) instead of DoubleRow's strided one.
    w1dr = nc.dram_tensor("w1dr", [128, NDR, 2 * CHID], fp8, kind="ExternalInput")
    w1r = nc.dram_tensor("w1r", [128, CHID], fp8, kind="ExternalInput")
    b1 = nc.dram_tensor("b1", [CHID, 1], f32, kind="ExternalInput")
    w2 = nc.dram_tensor("w2", [CHID, 1], bf16, kind="ExternalInput")
    b2 = nc.dram_tensor("b2", [128, 1], f32, kind="ExternalInput")  # replicated
    y = nc.dram_tensor("y", [BLOC, H, W], f32, kind="ExternalOutput")

    with tile.TileContext(nc) as tc:
        with (
            tc.tile_pool(name="consts", bufs=1) as consts,
            tc.tile_pool(name="xin", bufs=3 if reload else 1) as xin,
            tc.tile_pool(name="hbuf", bufs=6) as hbuf,
            tc.tile_pool(name="obuf", bufs=2) as obuf,
            tc.tile_pool(name="ogbuf", bufs=2) as ogbuf,
            tc.tile_pool(name="ps1", bufs=2, space="PSUM") as ps1,
            tc.tile_pool(name="ps2", bufs=2, space="PSUM") as ps2,
            tc.tile_pool(name="ps3", bufs=2, space="PSUM") as ps3,
        ):
            # Constants: conv weights + biases, resident for the whole kernel
            w1dr_s = consts.tile([128, NDR, 2 * CHID], fp8)
            nc.sync.dma_start(out=w1dr_s[:], in_=w1dr[:])
            w1r_s = consts.tile([128, CHID], fp8)
            nc.sync.dma_start(out=w1r_s, in_=w1r[:])
            b1_s = consts.tile([CHID, 1], f32)
            nc.sync.dma_start(out=b1_s, in_=b1[:])
            w2_s = consts.tile([CHID, 1], bf16)
            nc.sync.dma_start(out=w2_s, in_=w2[:])
            b2_s = consts.tile([128, 1], f32)
            nc.sync.dma_start(out=b2_s, in_=b2[:])

            # conv2 for full tiles runs as a col-tiled pack: 4 consecutive
            # tiles' 1x1 matmuls issued back-to-back at tile_position
            # (0, 32j) execute concurrently in distinct 32-column groups of
            # the PE array (one ~512-cycle span instead of four).  Sigmoid
            # runs once over the whole PSUM bank (rows 32j hold real data),
            # then a partition-gather DMA packs rows {0,32,64,96} into the
            # single-partition out_s staging row.  The 126-col tail tile
            # keeps the simple one-tile-lag path.
            def pack_flush(hlist, out_p, gcol):
                p2c = ps2.tile([128, TS], f32)
                for j, h_p in enumerate(hlist):
                    nc.tensor.matmul(out=p2c[32 * j:32 * j + 1, :],
                                     lhsT=w2_s[:], rhs=h_p[:, :TS],
                                     start=True, stop=True,
                                     tile_position=(0, 32 * j))
                og = ogbuf.tile([128, TS], f32)
                nc.scalar.activation(
                    out=og[:], in_=p2c[:],
                    func=mybir.ActivationFunctionType.Sigmoid,
                    bias=b2_s[:], scale=1.0,
                )
                src = og[:].copy()
                v = src.ap
                v[0] = (32 * TS, 4)
                src.ap = v
                dst = out_p[0:1, gcol:gcol + 4 * TS]
                nc.sync.dma_start(
                    out=dst.rearrange("p (a b) -> p a b", a=4), in_=src)

            pend = None  # tail tile: (h_s, out_s, col, n, img)

            def flush(pend):
                h_p, out_p, col, n, img = pend
                p2 = ps3.tile([1, TS], f32)
                nc.tensor.matmul(out=p2[0:1, :n], lhsT=w2_s[:], rhs=h_p[:, :n],
                                 start=True, stop=True)
                nc.scalar.activation(
                    out=out_p[0:1, col:col + n], in_=p2[0:1, :n],
                    func=mybir.ActivationFunctionType.Sigmoid,
                    bias=b2_s[0:1], scale=1.0,
                )
                if img is not None:
                    # Image done: write back the valid 64x64 pixels. Padded
                    # idx of (h,w) is START + 66*h + w -> out_s col 66*h + w.
                    src = out_p[0:1, :OUTW].rearrange("p (h w) -> p h w", w=WP)
                    nc.sync.dma_start(out=y[img], in_=src[:, :, 0:W])

            preloaded = {}
            if not reload:  # benchmark mode: load all images once up front
                for i in range(BLOC):
                    xc = xin.tile([128, XTOT], fp8, tag=f"xp{i}")
                    nc.sync.dma_start(out=xc[:], in_=x[i])
                    preloaded[i] = xc

            for i in [ii for _ in range(reps) for ii in range(BLOC)]:
                if not reload:
                    xs = preloaded[i]
                else:
                    xs = xin.tile([128, XTOT], fp8, tag="xs")
                    # Interleave region segments across both HWDGE queues so
                    # the early columns of every region land first: tile 0
                    # needs the head of ch0/ch1/ch2/ch2' all at once.
                    dma_eng = [nc.sync, nc.scalar, nc.sync]
                    nseg = 3
                    seg = -(-S // nseg)
                    for g in range(nseg):
                        a = g * seg
                        for r, (rb, rw) in enumerate(
                                [(0, S), (S, S), (2 * S, S), (C2B, NPIX)]):
                            b_ = min(a + seg, rw)
                            if a >= b_:
                                continue
                            eng = dma_eng[(g * 4 + r) % len(dma_eng)]
                            eng.dma_start(out=xs[:, rb + a:rb + b_],
                                          in_=x[i, :, rb + a:rb + b_])

                out_s = obuf.tile([1, OUTW], f32)
                group_h = []
                for t in range(NT):
                    n = TILE_N[t]
                    s0 = START + t * TS
                    p1 = ps1.tile([CHID, TS], f32)
                    for u, (ubase, ustep) in enumerate(DR_UNITS):
                        nc.tensor.matmul(
                            out=p1[:, :n],
                            lhsT=w1dr_s[:, u],
                            rhs=_pair_ap(xs, ubase + s0, ustep, n),
                            start=(u == 0),
                            stop=False,
                            perf_mode=mybir.MatmulPerfMode.DoubleRowSwInterleave,
                        )
                    nc.tensor.matmul(
                        out=p1[:, :n],
                        lhsT=w1r_s[:],
                        rhs=xs[:, REG_BASE + s0:REG_BASE + s0 + n],
                        start=False,
                        stop=True,
                    )
                    # Emit deferred conv2 work after this tile's conv1 MMs so
                    # the PE never waits on the ACT ReLU that produces h.
                    if t == 4 or t == 8:
                        pack_flush(group_h, out_s, (t - 4) * TS)
                        group_h = []
                    if t == 0 and pend is not None:
                        flush(pend)
                    # h = relu(conv + b1), rounded to bf16 for the 1x1 matmul
                    h_s = hbuf.tile([CHID, TS], bf16)
                    nc.scalar.activation(
                        out=h_s[:, :n], in_=p1[:, :n],
                        func=mybir.ActivationFunctionType.Relu,
                        bias=b1_s[:], scale=1.0,
                    )
                    if t == NT - 1:
                        pend = (h_s, out_s, t * TS, n, i)
                    else:
                        group_h.append(h_s)
            flush(pend)
    nc.compile()
    return nc


def _prep_inputs(dino_features, W1, b1, W2, b2):
    xp = np.zeros((B, HP, WP, CIN), dtype=np.float32)
    xp[:, 1:H + 1, 1:W + 1, :] = dino_features
    # -> [B, cin, padded_pixel]
    xt = xp.transpose(0, 3, 1, 2).reshape(B, CIN, NPIX).astype(E4M3)
    xbuf = np.zeros((B, 128, XTOT), dtype=E4M3)
    for c in range(3):
        xbuf[:, :, c * S:c * S + NPIX] = xt[:, c * 128:(c + 1) * 128]
    xbuf[:, :, C2B:C2B + NPIX] = xt[:, 256:384]

    # W1 (3,3,384,128) (ky,kx,ci,co); tap t=ky*3+kx pairs per DR_UNITS order.
    w1q = W1.astype(E4M3)
    pairs = np.zeros((128, NDR, 2, CHID), dtype=E4M3)
    for t in range(9):
        ky, kx = t // 3, t % 3
        pairs[:, t, 0] = w1q[ky, kx, 0:128]
        pairs[:, t, 1] = w1q[ky, kx, 128:256]
    for i in range(3):
        pairs[:, 9 + i, 0] = w1q[0, i, 256:384]
        pairs[:, 9 + i, 1] = w1q[1, i, 256:384]
    pairs[:, 12, 0] = w1q[2, 0, 256:384]
    pairs[:, 12, 1] = w1q[2, 2, 256:384]
    # SwInterleave weight bytes: w[p, u, 2*(127-m)+i] = pair_i[p, m]
    w1dr_h = np.zeros((128, NDR, 2 * CHID), dtype=E4M3)
    w1dr_h[:, :, 0::2] = pairs[:, :, 0, ::-1]
    w1dr_h[:, :, 1::2] = pairs[:, :, 1, ::-1]
    w1r_h = np.ascontiguousarray(w1q[2, 1, 256:384])

    b1h = np.ascontiguousarray(b1.reshape(CHID, 1).astype(np.float32))
    w2h = np.ascontiguousarray(W2.reshape(CHID, 1).astype(BF16))
    b2h = np.ascontiguousarray(
        np.broadcast_to(b2.reshape(1, 1), (128, 1)).astype(np.float32))

    in_maps = []
    for c in range(NCORES):
        in_maps.append({
            "x": np.ascontiguousarray(xbuf[c * BLOC:(c + 1) * BLOC]),
            "w1dr": w1dr_h, "w1r": w1r_h, "b1": b1h, "w2": w2h, "b2": b2h,
        })
    return in_maps


def kernel(dino_features, W1, b1, W2, b2, _trace=False, _trace_kwargs=None):
    dino_features = np.asarray(dino_features, dtype=np.float32)
    W1 = np.asarray(W1, dtype=np.float32)
    b1 = np.asarray(b1, dtype=np.float32)
    W2 = np.asarray(W2, dtype=np.float32)
    b2 = np.asarray(b2, dtype=np.float32)
    if "nc" not in _CACHED:
        _CACHED["nc"] = _build_bass()
    nc = _CACHED["nc"]
    in_maps = _prep_inputs(dino_features, W1, b1, W2, b2)
    res = run_bass_kernel_spmd(nc, in_maps, core_ids=list(range(NCORES)),
                               trace=_trace, **(_trace_kwargs or {}))
    _CACHED["last_results"] = res
    out = np.concatenate([res.results[c]["y"] for c in range(NCORES)], axis=0)
    return out.reshape(B, H, W, 1).astype(np.float32)
